# revision 49
# baseline (speedup 1.0000x reference)
"""Trainium2 Bass kernel for nn_Block_66425964200172 (dense transformer block).

Returns (x_out [2,2048,1024] f32, attn [2,16,2048,2048] f32) matching reference.

Sharding: sequence-parallel. 8 cores; core c handles batch b=c//4, query rows
[512*(c%4), 512*(c%4)+512). Each core computes LN1 + K/V for its whole batch
(K/V duplicated within the 4-core batch group -- cheaper than a collective),
attention for all 16 heads on its 512 query rows, then proj/LN2/MLP
token-parallel on those rows. Zero collectives.

Precision: float32r matmuls (full-speed fp32, ~1e-4 relative), bf16 only for
the P@V contraction and fc2 (error lands on x_out, not the attn output).
All LN scales/biases and qkv/fc1 biases are folded on the host into weights /
per-partition eviction biases. mask+attention_bias are folded into a single
additive `addend` injected into PSUM via an identity matmul.
"""
import os
import sys

for _p in ("/opt/trn_rl_repo", "/root/.axon_site/_ro/trn_rl_repo"):
    if os.path.isdir(_p) and _p not in sys.path:
        sys.path.insert(0, _p)
        break

from contextlib import ExitStack

import ml_dtypes
import numpy as np

import concourse.bacc as bacc
import concourse.mybir as mybir
import concourse.tile as tile
from concourse import bass_utils
from concourse.masks import make_identity

F32 = mybir.dt.float32
F32R = mybir.dt.float32r
BF16 = mybir.dt.bfloat16
AF = mybir.ActivationFunctionType
OP = mybir.AluOpType

B, N, C, H, HD, HID = 2, 2048, 1024, 16, 64, 4096
P = 128
QR = 512            # query rows per core
NCORES = 8
SCALE = HD ** -0.5
MASK_VAL = -65504.0
EPS = 1e-5

_CACHED_NC = None
PHASE_MARKS = []


def _ln_stats_apply(nc, lnp, eps_t, x_ap, out_ap):
    """LayerNorm core (center+scale only) of x_ap [128, 1024] -> out_ap."""
    st_ = lnp.tile([P, 2, 6], F32, tag="st")
    xr = x_ap.rearrange("p (a b) -> p a b", a=2)
    nc.vector.bn_stats(st_[:, 0, :], xr[:, 0, :])
    nc.vector.bn_stats(st_[:, 1, :], xr[:, 1, :])
    mv = lnp.tile([P, 2], F32, tag="mv")
    nc.vector.bn_aggr(mv[:], st_[:])
    nc.scalar.activation(mv[:, 1:2], mv[:, 1:2], AF.Sqrt, bias=eps_t[:])
    nc.vector.reciprocal(mv[:, 1:2], mv[:, 1:2])
    nc.vector.tensor_scalar(
        out=out_ap, in0=x_ap, scalar1=mv[:, 0:1], scalar2=mv[:, 1:2],
        op0=OP.subtract, op1=OP.mult,
    )


def build():
    PHASE_MARKS.clear()

    def mark(label):
        pass  # patched below once nc exists

    nc = bacc.Bacc("TRN2", target_bir_lowering=False, debug=False,
                   num_devices=NCORES)

    xb_d = nc.dram_tensor("xb", [N, C], F32, kind="ExternalInput").ap()
    xq_d = nc.dram_tensor("xq", [QR, C], F32, kind="ExternalInput").ap()
    ad_d = nc.dram_tensor("addend", [QR, N], F32R, kind="ExternalInput").ap()
    wq_d = nc.dram_tensor("wqT", [C, C], F32R, kind="ExternalInput").ap()
    wk_d = nc.dram_tensor("wkT", [C, C], F32R, kind="ExternalInput").ap()
    wv_d = nc.dram_tensor("wvT", [C, C], F32R, kind="ExternalInput").ap()
    pj_d = nc.dram_tensor("projT", [C, C], F32R, kind="ExternalInput").ap()
    w1_d = nc.dram_tensor("w1T", [C, HID], F32R, kind="ExternalInput").ap()
    w2_d = nc.dram_tensor("w2T", [HID, C], BF16, kind="ExternalInput").ap()
    qb_d = nc.dram_tensor("qb_c", [P, 8], F32, kind="ExternalInput").ap()
    kb_d = nc.dram_tensor("kb_c", [P, 8], F32, kind="ExternalInput").ap()
    vb_d = nc.dram_tensor("vb_c", [P, 8], F32, kind="ExternalInput").ap()
    pb_d = nc.dram_tensor("pb_c", [P, 8], F32, kind="ExternalInput").ap()
    f1b_d = nc.dram_tensor("f1b_c", [P, 32], F32, kind="ExternalInput").ap()
    f2b_d = nc.dram_tensor("f2b_c", [P, 8], F32, kind="ExternalInput").ap()
    attn_d = nc.dram_tensor("attn_o", [H, QR, N], F32, kind="ExternalOutput").ap()
    xo_d = nc.dram_tensor("x_o", [QR, C], F32, kind="ExternalOutput").ap()

    def mark(label):
        PHASE_MARKS.append((label, int(nc.next_id())))

    with tile.TileContext(nc) as tc, ExitStack() as top:
        pers = top.enter_context(tc.tile_pool(name="pers", bufs=1))
        ps_w = top.enter_context(tc.tile_pool(name="ps_w", bufs=1, space="PSUM"))

        ident_f32 = pers.tile([P, P], F32)
        make_identity(nc, ident_f32[:])
        ident_bf16 = pers.tile([P, P], BF16)
        nc.scalar.copy(ident_bf16[:], ident_f32[:])
        ident_f32r = pers.tile([P, P], F32R)
        nc.scalar.copy(ident_f32r[:], ident_f32[:])
        eps_t = pers.tile([P, 1], F32)
        nc.gpsimd.memset(eps_t[:], EPS)

        qb_sb = pers.tile([P, 8], F32)
        kb_sb = pers.tile([P, 8], F32)
        vb_sb = pers.tile([P, 8], F32)
        pb_sb = pers.tile([P, 8], F32)
        f2b_sb = pers.tile([P, 8], F32)
        f1b_sb = pers.tile([P, 32], F32)
        for sb_t, d_ap in ((qb_sb, qb_d), (kb_sb, kb_d), (vb_sb, vb_d),
                           (pb_sb, pb_d), (f2b_sb, f2b_d), (f1b_sb, f1b_d)):
            nc.sync.dma_start(sb_t[:], d_ap)

        ctxT = pers.tile([P, 8, QR], F32R)   # ctx^T: [c=(h,d), q]

        with ExitStack() as s_attn:
            hT_p = s_attn.enter_context(tc.tile_pool(name="hTp", bufs=1))
            hT0 = hT_p.tile([P, 8, 512], F32R, tag="hT0")
            hT1 = hT_p.tile([P, 8, 512], F32R, tag="hT1")
            hT2 = hT_p.tile([P, 8, 512], F32R, tag="hT2")
            hT3 = hT_p.tile([P, 8, 512], F32R, tag="hT3")
            hTs = [hT0, hT1, hT2, hT3]           # LN1(xb)^T: [c, tok] chunks
            qT = hT_p.tile([P, 8, QR], F32R)     # q_eff^T: [ch, qrow]

            with ExitStack() as s_ln:
                lnp = s_ln.enter_context(tc.tile_pool(name="lnp", bufs=5))
                ps_ln = s_ln.enter_context(
                    tc.tile_pool(name="ps_ln", bufs=3, space="PSUM"))

                def ln_to_T(src_dram, n_rows, dst_tiles):
                    # dst_tiles: list of [P, 8, 512] chunk tiles along tokens
                    for rt in range(n_rows // P):
                        xt = lnp.tile([P, C], F32, tag="xt")
                        nc.sync.dma_start(xt[:], src_dram[P * rt:P * rt + P, :])
                        hn = lnp.tile([P, C], F32, tag="hn")
                        _ln_stats_apply(nc, lnp, eps_t, xt[:], hn[:])
                        tp = ps_ln.tile([P, 1024], F32, tag="ln")
                        for ct in range(8):
                            nc.tensor.transpose(
                                tp[:, P * ct:P * ct + P],
                                hn[:, P * ct:P * ct + P], ident_f32[:])
                        dstT = dst_tiles[rt // 4]
                        ro = (rt % 4) * P
                        nc.scalar.copy(
                            dstT[:, 0:8, ro:ro + P],
                            tp[:].rearrange("p (a b) -> p a b", b=P))

                mark('q_path')
                with ExitStack() as s_q:
                    hq_p = s_q.enter_context(tc.tile_pool(name="hqp", bufs=1))
                    ps_q = s_q.enter_context(
                        tc.tile_pool(name="ps_q", bufs=1, space="PSUM"))
                    hqTt = hq_p.tile([P, 8, QR], F32R)
                    hqT = hqTt
                    wqf = hq_p.tile([P, 8, C], F32R)
                    ln_to_T(xq_d, QR, [hqT])
                    for ct in range(8):
                        nc.sync.dma_start(wqf[:, ct, :], wq_d[P * ct:P * ct + P, :])
                    for ct2 in range(8):
                        qp = ps_q.tile([P, 512], F32, tag="qp")
                        for ct in range(8):
                            nc.tensor.matmul(
                                qp[:], wqf[:, ct, P * ct2:P * ct2 + P],
                                hqT[:, ct, :], start=(ct == 0), stop=(ct == 7))
                        nc.scalar.activation(qT[:, ct2, :], qp[:], AF.Identity,
                                             bias=qb_sb[:, ct2:ct2 + 1], scale=SCALE)
                    mark('ln_xb')
                    ln_to_T(xb_d, N, hTs)

            mark('attn')
            with ExitStack() as s_at:
                kv_p = s_at.enter_context(tc.tile_pool(name="kvp", bufs=1))
                pex_p = s_at.enter_context(tc.tile_pool(name="pex", bufs=4))
                pbf_p = s_at.enter_context(tc.tile_pool(name="pbf", bufs=4))
                pet_p = s_at.enter_context(tc.tile_pool(name="pet", bufs=3))
                rs_p = s_at.enter_context(tc.tile_pool(name="rsp", bufs=3))
                ps_sq = s_at.enter_context(
                    tc.tile_pool(name="ps_sq", bufs=2, space="PSUM"))
                ps_t = s_at.enter_context(
                    tc.tile_pool(name="ps_t", bufs=2, space="PSUM"))
                ps_pv = s_at.enter_context(
                    tc.tile_pool(name="ps_pv", bufs=1, space="PSUM"))

                addend_sb = kv_p.tile([P, 4, N], F32R, tag="ad")
                for qt in range(4):
                    nc.sync.dma_start(addend_sb[:, qt, :], ad_d[P * qt:P * qt + P, :])

                for g in range(4):
                    mark(f'kv_g{g}')
                    kT_g = kv_p.tile([P, 2, N], F32R, tag="ktg")
                    v_g = kv_p.tile([P, 16, 256], BF16, tag="vg")
                    wk_g = kv_p.tile([P, 8, 256], F32R, tag="wkv")
                    with (tc.high_priority() if g == 0 else ExitStack()):
                        for ct in range(8):
                            nc.sync.dma_start(
                                wk_g[:, ct, :],
                                wk_d[P * ct:P * ct + P, 256 * g:256 * g + 256])
                    for j in range(2):
                        cix = 2 * g + j
                        for ch in range(4):
                            kp = ps_w.tile([P, 512], F32, tag="w")
                            for ct in range(8):
                                nc.tensor.matmul(
                                    kp[:], wk_g[:, ct, P * j:P * j + P],
                                    hTs[ch][:, ct, :],
                                    start=(ct == 0), stop=(ct == 7))
                            nc.vector.tensor_scalar_add(
                                kT_g[:, j, 512 * ch:512 * ch + 512], kp[:],
                                kb_sb[:, cix:cix + 1])
                    wv_g = kv_p.tile([P, 8, 256], F32R, tag="wkv")
                    for ct in range(8):
                        nc.sync.dma_start(
                            wv_g[:, ct, :], wv_d[P * ct:P * ct + P, 256 * g:256 * g + 256])
                    for tt in range(16):
                        vp = ps_w.tile([P, 256], F32, tag="w")
                        for ct in range(8):
                            nc.tensor.matmul(
                                vp[:], hTs[tt // 4][:, ct, (tt % 4) * P:(tt % 4) * P + P],
                                wv_g[:, ct, :],
                                start=(ct == 0), stop=(ct == 7))
                        nc.vector.tensor_copy(v_g[:, tt, :], vp[:])

                    mark(f'at_g{g}')
                    for hp in range(2):
                        pv = ps_pv.tile([P, QR], F32)
                        for sub in range(2):
                            hl = 2 * hp + sub
                            hh = 4 * g + hl
                            for qt in range(4):
                                pet = pet_p.tile([P, 16, P], BF16, tag="pet")
                                half0 = pex_p.tile([P, 1024], F32, tag="pe")
                                half1 = pex_p.tile([P, 1024], F32, tag="pe")
                                halves = [half0, half1]
                                rs = rs_p.tile([P, 2], F32, tag="rs")
                                for hf in range(2):
                                    sp = ps_sq.tile([P, 1024], F32, tag="s")
                                    for kc in range(2):
                                        ko = 1024 * hf + 512 * kc
                                        sl = sp[:, 512 * kc:512 * kc + 512]
                                        nc.tensor.matmul(
                                            sl,
                                            qT[64 * (hh % 2):64 * (hh % 2) + 64,
                                               hh // 2, P * qt:P * qt + P],
                                            kT_g[64 * (hl % 2):64 * (hl % 2) + 64,
                                                 hl // 2, ko:ko + 512],
                                            start=True, stop=False)
                                        nc.tensor.matmul(
                                            sl, ident_f32r[:],
                                            addend_sb[:, qt, ko:ko + 512],
                                            start=False, stop=True)
                                    nc.scalar.activation(
                                        halves[hf][:], sp[:],
                                        AF.Exp, accum_out=rs[:, hf:hf + 1])
                                rcp = rs_p.tile([P, 1], F32, tag="rcp")
                                nc.vector.reduce_sum(rcp[:], rs[:],
                                                     axis=mybir.AxisListType.X)
                                nc.vector.reciprocal(rcp[:], rcp[:])
                                P_bf = pbf_p.tile([P, N], BF16, tag="pb")
                                for hf in range(2):
                                    nc.gpsimd.tensor_scalar_mul(
                                        P_bf[:, 1024 * hf:1024 * hf + 1024],
                                        halves[hf][:], rcp[:])
                                    nc.vector.tensor_scalar_mul(
                                        halves[hf][:], halves[hf][:], rcp[:])
                                    nc.sync.dma_start(
                                        attn_d[hh, P * qt:P * qt + P,
                                               1024 * hf:1024 * hf + 1024],
                                        halves[hf][:])
                                for jb in range(4):
                                    tp = ps_t.tile([P, 512], BF16, tag="t")
                                    for j4 in range(4):
                                        j = 4 * jb + j4
                                        nc.tensor.transpose(
                                            tp[:, P * j4:P * j4 + P],
                                            P_bf[:, P * j:P * j + P], ident_bf16[:])
                                    dst = pet[:, 4 * jb:4 * jb + 4, :]
                                    src = tp[:].rearrange("p (a b) -> p a b", b=P)
                                    if jb == 0:
                                        nc.scalar.copy(dst, src)
                                    else:
                                        nc.vector.tensor_copy(dst, src)
                                # incremental PV on this qt column
                                for kt in range(16):
                                    nc.tensor.matmul(
                                        pv[64 * sub:64 * sub + 64, P * qt:P * qt + P],
                                        v_g[:, kt, 64 * hl:64 * hl + 64],
                                        pet[:, kt, :],
                                        start=(kt == 0), stop=(kt == 15))
                        nc.scalar.activation(
                            ctxT[:, 2 * g + hp, :], pv[:], AF.Identity,
                            bias=vb_sb[:, 2 * g + hp:2 * g + hp + 1], scale=1.0)

        mark('mlp')
        with ExitStack() as s_m:
            big = s_m.enter_context(tc.tile_pool(name="big", bufs=1))
            y2_p = s_m.enter_context(tc.tile_pool(name="y2p", bufs=2))
            lnm = s_m.enter_context(tc.tile_pool(name="lnm", bufs=4))
            ps_m = s_m.enter_context(tc.tile_pool(name="ps_m", bufs=3, space="PSUM"))
            ps_tr = s_m.enter_context(
                tc.tile_pool(name="ps_tr", bufs=2, space="PSUM"))

            x2 = big.tile([P, 4, C], F32, tag="x2")
            xattnT = big.tile([P, 8, QR], F32, tag="xaT")
            h2T = big.tile([P, 8, QR], F32R, tag="h2T")
            gT = big.tile([P, 32, QR], BF16, tag="gT")
            xout = big.tile([P, 4, C], F32, tag="xout")

            # proj on ctxT (weights via a short-lived whole-load pool)
            with ExitStack() as s_pj:
                pjp = s_pj.enter_context(tc.tile_pool(name="pjp", bufs=1))
                pjf = pjp.tile([P, 8, C], F32R)
                for ct in range(8):
                    eng = nc.sync if ct % 2 == 0 else nc.scalar
                    eng.dma_start(pjf[:, ct, :], pj_d[P * ct:P * ct + P, :])
                for ct2 in range(8):
                    pp = ps_m.tile([P, 512], F32, tag="m")
                    for ct in range(8):
                        nc.tensor.matmul(pp[:], pjf[:, ct, P * ct2:P * ct2 + P],
                                         ctxT[:, ct, :], start=(ct == 0), stop=(ct == 7))
                    nc.scalar.activation(xattnT[:, ct2, :], pp[:], AF.Identity,
                                         bias=pb_sb[:, ct2:ct2 + 1], scale=1.0)
                # transpose xattnT + residual -> x2 (natural rows)
                for rt in range(4):
                    xqt = lnm.tile([P, C], F32, tag="xqt")
                    nc.sync.dma_start(xqt[:], xq_d[P * rt:P * rt + P, :])
                    tp = ps_tr.tile([P, 1024], F32, tag="tr")
                    for ct in range(8):
                        nc.tensor.transpose(tp[:, P * ct:P * ct + P],
                                            xattnT[:, ct, P * rt:P * rt + P],
                                            ident_f32[:])
                    nc.vector.tensor_add(x2[:, rt, :], tp[:], xqt[:])
                # LN2 -> h2T
                for rt in range(4):
                    h2n = lnm.tile([P, C], F32, tag="h2n")
                    _ln_stats_apply(nc, lnm, eps_t, x2[:, rt, :], h2n[:])
                    tp = ps_tr.tile([P, 1024], F32, tag="tr")
                    for ct in range(8):
                        nc.tensor.transpose(tp[:, P * ct:P * ct + P],
                                            h2n[:, P * ct:P * ct + P], ident_f32[:])
                    nc.scalar.copy(h2T[:, 0:8, P * rt:P * rt + P],
                                   tp[:].rearrange("p (a b) -> p a b", b=P))
            mark('fc1')
            # fc1 + gelu -> gT (bf16)
            with ExitStack() as s_f1:
                mw1 = s_f1.enter_context(tc.tile_pool(name="mw1", bufs=16))
                for hb in range(8):
                    w1c = []
                    for ct in range(8):
                        w = mw1.tile([P, 512], F32R, tag="w1c")
                        eng = nc.sync if ct % 2 == 0 else nc.scalar
                        eng.dma_start(
                            w[:], w1_d[P * ct:P * ct + P, 512 * hb:512 * hb + 512])
                        w1c.append(w)
                    for hq_ in range(4):
                        ht = 4 * hb + hq_
                        fp = ps_m.tile([P, 512], F32, tag="m")
                        for ct in range(8):
                            nc.tensor.matmul(
                                fp[:], w1c[ct][:, P * hq_:P * hq_ + P], h2T[:, ct, :],
                                start=(ct == 0), stop=(ct == 7))
                        nc.scalar.activation(gT[:, ht, :], fp[:], AF.Gelu,
                                             bias=f1b_sb[:, ht:ht + 1], scale=1.0)
            mark('fc2')
            # fc2 (bf16, half-resident weights) + residual
            with ExitStack() as s_f2:
                w2p = s_f2.enter_context(tc.tile_pool(name="w2p", bufs=1))
                y2a = s_f2.enter_context(tc.tile_pool(name="y2a", bufs=1))
                y2acc = y2a.tile([P, 8, QR], F32)
                for half in range(2):
                    w2h = w2p.tile([P, 16, C], BF16, tag="w2h")
                    for ht16 in range(16):
                        eng = nc.sync if ht16 % 2 == 0 else nc.scalar
                        eng.dma_start(
                            w2h[:, ht16, :],
                            w2_d[P * (16 * half + ht16):P * (16 * half + ht16) + P, :])
                    for ct2 in range(8):
                        fp = ps_m.tile([P, 512], F32, tag="m")
                        for ht16 in range(16):
                            nc.tensor.matmul(
                                fp[:], w2h[:, ht16, P * ct2:P * ct2 + P],
                                gT[:, 16 * half + ht16, :],
                                start=(ht16 == 0), stop=(ht16 == 15))
                        if half == 0:
                            nc.scalar.activation(
                                y2acc[:, ct2, :], fp[:], AF.Identity,
                                bias=f2b_sb[:, ct2:ct2 + 1], scale=1.0)
                        else:
                            y2t = y2_p.tile([P, QR], F32, tag="y2")
                            nc.vector.tensor_add(y2t[:], fp[:], y2acc[:, ct2, :])
                            tp = ps_tr.tile([P, 512], F32, tag="tr")
                            for rt in range(4):
                                nc.tensor.transpose(
                                    tp[:, P * rt:P * rt + P],
                                    y2t[:, P * rt:P * rt + P], ident_f32[:])
                            nc.vector.tensor_add(
                                xout[:, 0:4, P * ct2:P * ct2 + P],
                                tp[:].rearrange("p (a b) -> p a b", b=P),
                                x2[:, 0:4, P * ct2:P * ct2 + P])
            mark('end')
            for rt in range(4):
                nc.sync.dma_start(xo_d[P * rt:P * rt + P, :], xout[:, rt, :])

    nc.compile()
    return nc


def _get_nc():
    global _CACHED_NC
    if _CACHED_NC is None:
        _CACHED_NC = build()
    return _CACHED_NC


def _cols(v, width):
    return np.ascontiguousarray(np.asarray(v, np.float32).reshape(-1, P).T)


def kernel(x, attention_mask, attention_bias, qkv_w, q_bias, v_bias,
           proj_w, proj_b, ln1_s, ln1_b, ln2_s, ln2_b,
           fc1_w, fc1_b, fc2_w, fc2_b):
    f32 = lambda a: np.ascontiguousarray(np.asarray(a, np.float32))
    x = f32(x)
    mask = np.asarray(attention_mask, bool)
    bias_full = f32(attention_bias)
    qkv_w, proj_w, fc1_w, fc2_w = map(f32, (qkv_w, proj_w, fc1_w, fc2_w))
    q_bias, v_bias, proj_b, fc1_b, fc2_b = map(
        f32, (q_bias, v_bias, proj_b, fc1_b, fc2_b))
    ln1_s, ln1_b, ln2_s, ln2_b = map(f32, (ln1_s, ln1_b, ln2_s, ln2_b))

    wq, wk, wv = qkv_w[0:C], qkv_w[C:2 * C], qkv_w[2 * C:3 * C]
    wqT = np.ascontiguousarray((wq * ln1_s[None, :]).T)
    wkT = np.ascontiguousarray((wk * ln1_s[None, :]).T)
    wvT = np.ascontiguousarray((wv * ln1_s[None, :]).T)
    projT = np.ascontiguousarray(proj_w.T)
    w1T = np.ascontiguousarray((fc1_w * ln2_s[None, :]).T)
    w2T = np.ascontiguousarray(fc2_w.T).astype(ml_dtypes.bfloat16)

    qb = (q_bias + wq @ ln1_b) * np.float32(SCALE)
    kb = wk @ ln1_b
    vb = v_bias + wv @ ln1_b
    f1b = fc1_b + fc1_w @ ln2_b

    addend = (bias_full[None, :, :]
              - np.float32(-MASK_VAL) * mask[:, None, :].astype(np.float32))
    addend = np.ascontiguousarray(addend, np.float32)

    shared = {
        "wqT": wqT, "wkT": wkT, "wvT": wvT, "projT": projT,
        "w1T": w1T, "w2T": w2T,
        "qb_c": _cols(qb, 8), "kb_c": _cols(kb, 8), "vb_c": _cols(vb, 8),
        "pb_c": _cols(proj_b, 8), "f1b_c": _cols(f1b, 32),
        "f2b_c": _cols(fc2_b, 8),
    }
    in_maps = []
    for c in range(NCORES):
        b, s = c // 4, c % 4
        rows = slice(QR * s, QR * s + QR)
        in_maps.append(dict(
            shared,
            xb=np.ascontiguousarray(x[b]),
            xq=np.ascontiguousarray(x[b, rows]),
            addend=np.ascontiguousarray(addend[b, rows]),
        ))

    nc = _get_nc()
    res = bass_utils.run_bass_kernel_spmd(nc, in_maps, core_ids=list(range(NCORES)))

    x_out = np.empty((B, N, C), np.float32)
    attn = np.empty((B, H, N, N), np.float32)
    for c in range(NCORES):
        b, s = c // 4, c % 4
        rows = slice(QR * s, QR * s + QR)
        x_out[b, rows] = res.results[c]["x_o"]
        attn[b, :, rows, :] = res.results[c]["attn_o"]
    return x_out, attn


# revision 52
# speedup vs baseline: 1.0124x; 1.0124x over previous
"""Trainium2 Bass kernel for nn_Block_66425964200172 (dense transformer block).

Returns (x_out [2,2048,1024] f32, attn [2,16,2048,2048] f32) matching reference.

Sharding: sequence-parallel. 8 cores; core c handles batch b=c//4, query rows
[512*(c%4), 512*(c%4)+512). Each core computes LN1 + K/V for its whole batch
(K/V duplicated within the 4-core batch group -- cheaper than a collective),
attention for all 16 heads on its 512 query rows, then proj/LN2/MLP
token-parallel on those rows. Zero collectives.

Precision: float32r matmuls (full-speed fp32, ~1e-4 relative), bf16 only for
the P@V contraction and fc2 (error lands on x_out, not the attn output).
All LN scales/biases and qkv/fc1 biases are folded on the host into weights /
per-partition eviction biases. mask+attention_bias are folded into a single
additive `addend` injected into PSUM via an identity matmul.
"""
import os
import sys

for _p in ("/opt/trn_rl_repo", "/root/.axon_site/_ro/trn_rl_repo"):
    if os.path.isdir(_p) and _p not in sys.path:
        sys.path.insert(0, _p)
        break

from contextlib import ExitStack

import ml_dtypes
import numpy as np

import concourse.bacc as bacc
import concourse.mybir as mybir
import concourse.tile as tile
from concourse import bass_utils
from concourse.masks import make_identity

F32 = mybir.dt.float32
F32R = mybir.dt.float32r
BF16 = mybir.dt.bfloat16
AF = mybir.ActivationFunctionType
OP = mybir.AluOpType

B, N, C, H, HD, HID = 2, 2048, 1024, 16, 64, 4096
P = 128
QR = 512            # query rows per core
NCORES = 8
SCALE = HD ** -0.5
MASK_VAL = -65504.0
EPS = 1e-5

_CACHED_NC = None
PHASE_MARKS = []


def _ln_stats_apply(nc, lnp, eps_t, x_ap, out_ap):
    """LayerNorm core (center+scale only) of x_ap [128, 1024] -> out_ap."""
    st_ = lnp.tile([P, 2, 6], F32, tag="st")
    xr = x_ap.rearrange("p (a b) -> p a b", a=2)
    nc.vector.bn_stats(st_[:, 0, :], xr[:, 0, :])
    nc.vector.bn_stats(st_[:, 1, :], xr[:, 1, :])
    mv = lnp.tile([P, 2], F32, tag="mv")
    nc.vector.bn_aggr(mv[:], st_[:])
    nc.scalar.activation(mv[:, 1:2], mv[:, 1:2], AF.Sqrt, bias=eps_t[:])
    nc.vector.reciprocal(mv[:, 1:2], mv[:, 1:2])
    nc.vector.tensor_scalar(
        out=out_ap, in0=x_ap, scalar1=mv[:, 0:1], scalar2=mv[:, 1:2],
        op0=OP.subtract, op1=OP.mult,
    )


def build():
    PHASE_MARKS.clear()

    def mark(label):
        pass  # patched below once nc exists

    nc = bacc.Bacc("TRN2", target_bir_lowering=False, debug=False,
                   num_devices=NCORES)

    xb_d = nc.dram_tensor("xb", [N, C], F32, kind="ExternalInput").ap()
    xq_d = nc.dram_tensor("xq", [QR, C], F32, kind="ExternalInput").ap()
    ad_d = nc.dram_tensor("addend", [QR, N], F32R, kind="ExternalInput").ap()
    wq_d = nc.dram_tensor("wqT", [C, C], F32R, kind="ExternalInput").ap()
    wk_d = nc.dram_tensor("wkT", [C, C], F32R, kind="ExternalInput").ap()
    wv_d = nc.dram_tensor("wvT", [C, C], F32R, kind="ExternalInput").ap()
    pj_d = nc.dram_tensor("projT", [C, C], F32R, kind="ExternalInput").ap()
    w1_d = nc.dram_tensor("w1T", [C, HID], F32R, kind="ExternalInput").ap()
    w2_d = nc.dram_tensor("w2T", [HID, C], BF16, kind="ExternalInput").ap()
    qb_d = nc.dram_tensor("qb_c", [P, 8], F32, kind="ExternalInput").ap()
    kb_d = nc.dram_tensor("kb_c", [P, 8], F32, kind="ExternalInput").ap()
    vb_d = nc.dram_tensor("vb_c", [P, 8], F32, kind="ExternalInput").ap()
    pb_d = nc.dram_tensor("pb_c", [P, 8], F32, kind="ExternalInput").ap()
    f1b_d = nc.dram_tensor("f1b_c", [P, 32], F32, kind="ExternalInput").ap()
    f2b_d = nc.dram_tensor("f2b_c", [P, 8], F32, kind="ExternalInput").ap()
    attn_d = nc.dram_tensor("attn_o", [H, QR, N], F32, kind="ExternalOutput").ap()
    xo_d = nc.dram_tensor("x_o", [QR, C], F32, kind="ExternalOutput").ap()

    def mark(label):
        PHASE_MARKS.append((label, int(nc.next_id())))

    with tile.TileContext(nc) as tc, ExitStack() as top:
        pers = top.enter_context(tc.tile_pool(name="pers", bufs=1))
        ps_w = top.enter_context(tc.tile_pool(name="ps_w", bufs=1, space="PSUM"))

        ident_f32 = pers.tile([P, P], F32)
        make_identity(nc, ident_f32[:])
        ident_bf16 = pers.tile([P, P], BF16)
        nc.scalar.copy(ident_bf16[:], ident_f32[:])
        ident_f32r = pers.tile([P, P], F32R)
        nc.scalar.copy(ident_f32r[:], ident_f32[:])
        eps_t = pers.tile([P, 1], F32)
        nc.gpsimd.memset(eps_t[:], EPS)

        qb_sb = pers.tile([P, 8], F32)
        kb_sb = pers.tile([P, 8], F32)
        vb_sb = pers.tile([P, 8], F32)
        pb_sb = pers.tile([P, 8], F32)
        f2b_sb = pers.tile([P, 8], F32)
        f1b_sb = pers.tile([P, 32], F32)
        for sb_t, d_ap in ((qb_sb, qb_d), (kb_sb, kb_d), (vb_sb, vb_d),
                           (pb_sb, pb_d), (f2b_sb, f2b_d), (f1b_sb, f1b_d)):
            nc.sync.dma_start(sb_t[:], d_ap)

        ctxT = pers.tile([P, 8, QR], F32R)   # ctx^T: [c=(h,d), q]

        with ExitStack() as s_attn:
            hT_p = s_attn.enter_context(tc.tile_pool(name="hTp", bufs=1))
            hT0 = hT_p.tile([P, 8, 512], F32R, tag="hT0")
            hT1 = hT_p.tile([P, 8, 512], F32R, tag="hT1")
            hT2 = hT_p.tile([P, 8, 512], F32R, tag="hT2")
            hT3 = hT_p.tile([P, 8, 512], F32R, tag="hT3")
            hTs = [hT0, hT1, hT2, hT3]           # LN1(xb)^T: [c, tok] chunks
            qT = hT_p.tile([P, 8, QR], F32R)     # q_eff^T: [ch, qrow]

            with ExitStack() as s_ln:
                lnp = s_ln.enter_context(tc.tile_pool(name="lnp", bufs=5))
                ps_ln = s_ln.enter_context(
                    tc.tile_pool(name="ps_ln", bufs=3, space="PSUM"))

                def ln_to_T(src_dram, n_rows, dst_tiles):
                    # dst_tiles: list of [P, 8, 512] chunk tiles along tokens
                    for rt in range(n_rows // P):
                        xt = lnp.tile([P, C], F32, tag="xt")
                        nc.sync.dma_start(xt[:], src_dram[P * rt:P * rt + P, :])
                        hn = lnp.tile([P, C], F32, tag="hn")
                        _ln_stats_apply(nc, lnp, eps_t, xt[:], hn[:])
                        tp = ps_ln.tile([P, 1024], F32, tag="ln")
                        for ct in range(8):
                            nc.tensor.transpose(
                                tp[:, P * ct:P * ct + P],
                                hn[:, P * ct:P * ct + P], ident_f32[:])
                        dstT = dst_tiles[rt // 4]
                        ro = (rt % 4) * P
                        nc.scalar.copy(
                            dstT[:, 0:8, ro:ro + P],
                            tp[:].rearrange("p (a b) -> p a b", b=P))

                mark('q_path')
                with ExitStack() as s_q:
                    hq_p = s_q.enter_context(tc.tile_pool(name="hqp", bufs=1))
                    ps_q = s_q.enter_context(
                        tc.tile_pool(name="ps_q", bufs=1, space="PSUM"))
                    hqTt = hq_p.tile([P, 8, QR], F32R)
                    hqT = hqTt
                    wqf = hq_p.tile([P, 8, C], F32R)
                    ln_to_T(xq_d, QR, [hqT])
                    for ct in range(8):
                        nc.sync.dma_start(wqf[:, ct, :], wq_d[P * ct:P * ct + P, :])
                    for ct2 in range(8):
                        qp = ps_q.tile([P, 512], F32, tag="qp")
                        for ct in range(8):
                            nc.tensor.matmul(
                                qp[:], wqf[:, ct, P * ct2:P * ct2 + P],
                                hqT[:, ct, :], start=(ct == 0), stop=(ct == 7))
                        nc.scalar.activation(qT[:, ct2, :], qp[:], AF.Identity,
                                             bias=qb_sb[:, ct2:ct2 + 1], scale=SCALE)
                    mark('ln_xb')
                    ln_to_T(xb_d, N, hTs)

            mark('attn')
            with ExitStack() as s_at:
                kv_p = s_at.enter_context(tc.tile_pool(name="kvp", bufs=1))
                pex_p = s_at.enter_context(tc.tile_pool(name="pex", bufs=4))
                pbf_p = s_at.enter_context(tc.tile_pool(name="pbf", bufs=4))
                pet_p = s_at.enter_context(tc.tile_pool(name="pet", bufs=3))
                rs_p = s_at.enter_context(tc.tile_pool(name="rsp", bufs=3))
                ps_sq = s_at.enter_context(
                    tc.tile_pool(name="ps_sq", bufs=2, space="PSUM"))
                ps_t = s_at.enter_context(
                    tc.tile_pool(name="ps_t", bufs=2, space="PSUM"))
                ps_pv = s_at.enter_context(
                    tc.tile_pool(name="ps_pv", bufs=1, space="PSUM"))

                addend_sb = kv_p.tile([P, 4, N], F32R, tag="ad")
                for qt in range(4):
                    nc.sync.dma_start(addend_sb[:, qt, :], ad_d[P * qt:P * qt + P, :])

                for g in range(4):
                    mark(f'kv_g{g}')
                    kT_g = kv_p.tile([P, 2, N], F32R, tag="ktg")
                    v_g = kv_p.tile([P, 16, 256], BF16, tag="vg")
                    wk_g = kv_p.tile([P, 8, 256], F32R, tag="wkv")
                    with (tc.high_priority() if g == 0 else ExitStack()):
                        for ct in range(8):
                            nc.sync.dma_start(
                                wk_g[:, ct, :],
                                wk_d[P * ct:P * ct + P, 256 * g:256 * g + 256])
                    for j in range(2):
                        cix = 2 * g + j
                        for ch in range(4):
                            kp = ps_w.tile([P, 512], F32, tag="w")
                            for ct in range(8):
                                nc.tensor.matmul(
                                    kp[:], wk_g[:, ct, P * j:P * j + P],
                                    hTs[ch][:, ct, :],
                                    start=(ct == 0), stop=(ct == 7))
                            nc.vector.tensor_scalar_add(
                                kT_g[:, j, 512 * ch:512 * ch + 512], kp[:],
                                kb_sb[:, cix:cix + 1])
                    wv_g = kv_p.tile([P, 8, 256], F32R, tag="wkv")
                    for ct in range(8):
                        nc.sync.dma_start(
                            wv_g[:, ct, :], wv_d[P * ct:P * ct + P, 256 * g:256 * g + 256])
                    for tt in range(16):
                        vp = ps_w.tile([P, 256], F32, tag="w")
                        for ct in range(8):
                            nc.tensor.matmul(
                                vp[:], hTs[tt // 4][:, ct, (tt % 4) * P:(tt % 4) * P + P],
                                wv_g[:, ct, :],
                                start=(ct == 0), stop=(ct == 7))
                        nc.vector.tensor_copy(v_g[:, tt, :], vp[:])

                    mark(f'at_g{g}')
                    for hp in range(2):
                        pv = ps_pv.tile([P, QR], F32)
                        for sub in range(2):
                            hl = 2 * hp + sub
                            hh = 4 * g + hl
                            for qt in range(4):
                                pet = pet_p.tile([P, 16, P], BF16, tag="pet")
                                half0 = pex_p.tile([P, 1024], F32, tag="pe")
                                half1 = pex_p.tile([P, 1024], F32, tag="pe")
                                halves = [half0, half1]
                                rs = rs_p.tile([P, 2], F32, tag="rs")
                                for hf in range(2):
                                    sp = ps_sq.tile([P, 1024], F32, tag="s")
                                    for kc in range(2):
                                        ko = 1024 * hf + 512 * kc
                                        sl = sp[:, 512 * kc:512 * kc + 512]
                                        nc.tensor.matmul(
                                            sl,
                                            qT[64 * (hh % 2):64 * (hh % 2) + 64,
                                               hh // 2, P * qt:P * qt + P],
                                            kT_g[64 * (hl % 2):64 * (hl % 2) + 64,
                                                 hl // 2, ko:ko + 512],
                                            start=True, stop=False)
                                        nc.tensor.matmul(
                                            sl, ident_f32r[:],
                                            addend_sb[:, qt, ko:ko + 512],
                                            start=False, stop=True)
                                    nc.scalar.activation(
                                        halves[hf][:], sp[:],
                                        AF.Exp, accum_out=rs[:, hf:hf + 1])
                                rcp = rs_p.tile([P, 1], F32, tag="rcp")
                                nc.vector.reduce_sum(rcp[:], rs[:],
                                                     axis=mybir.AxisListType.X)
                                nc.vector.reciprocal(rcp[:], rcp[:])
                                P_bf = pbf_p.tile([P, N], BF16, tag="pb")
                                for hf in range(2):
                                    nc.gpsimd.tensor_scalar_mul(
                                        P_bf[:, 1024 * hf:1024 * hf + 1024],
                                        halves[hf][:], rcp[:])
                                    nc.vector.tensor_scalar_mul(
                                        halves[hf][:], halves[hf][:], rcp[:])
                                    nc.sync.dma_start(
                                        attn_d[hh, P * qt:P * qt + P,
                                               1024 * hf:1024 * hf + 1024],
                                        halves[hf][:])
                                for jb in range(2):
                                    tp = ps_t.tile([P, 1024], BF16, tag="t")
                                    for j4 in range(8):
                                        j = 8 * jb + j4
                                        nc.tensor.transpose(
                                            tp[:, P * j4:P * j4 + P],
                                            P_bf[:, P * j:P * j + P], ident_bf16[:])
                                    dst = pet[:, 8 * jb:8 * jb + 8, :]
                                    src = tp[:].rearrange("p (a b) -> p a b", b=P)
                                    if jb == 0:
                                        nc.scalar.copy(dst, src)
                                    else:
                                        nc.vector.tensor_copy(dst, src)
                                # incremental PV on this qt column
                                for kt in range(16):
                                    nc.tensor.matmul(
                                        pv[64 * sub:64 * sub + 64, P * qt:P * qt + P],
                                        v_g[:, kt, 64 * hl:64 * hl + 64],
                                        pet[:, kt, :],
                                        start=(kt == 0), stop=(kt == 15))
                        nc.scalar.activation(
                            ctxT[:, 2 * g + hp, :], pv[:], AF.Identity,
                            bias=vb_sb[:, 2 * g + hp:2 * g + hp + 1], scale=1.0)

        mark('mlp')
        with ExitStack() as s_m:
            big = s_m.enter_context(tc.tile_pool(name="big", bufs=1))
            y2_p = s_m.enter_context(tc.tile_pool(name="y2p", bufs=2))
            lnm = s_m.enter_context(tc.tile_pool(name="lnm", bufs=4))
            ps_m = s_m.enter_context(tc.tile_pool(name="ps_m", bufs=3, space="PSUM"))
            ps_tr = s_m.enter_context(
                tc.tile_pool(name="ps_tr", bufs=2, space="PSUM"))

            x2 = big.tile([P, 4, C], F32, tag="x2")
            xattnT = big.tile([P, 8, QR], F32, tag="xaT")
            h2T = big.tile([P, 8, QR], F32R, tag="h2T")
            gT = big.tile([P, 32, QR], BF16, tag="gT")
            xout = big.tile([P, 4, C], F32, tag="xout")

            # proj on ctxT (weights via a short-lived whole-load pool)
            with ExitStack() as s_pj:
                pjp = s_pj.enter_context(tc.tile_pool(name="pjp", bufs=1))
                pjf = pjp.tile([P, 8, C], F32R)
                for ct in range(8):
                    eng = nc.sync if ct % 2 == 0 else nc.scalar
                    eng.dma_start(pjf[:, ct, :], pj_d[P * ct:P * ct + P, :])
                for ct2 in range(8):
                    pp = ps_m.tile([P, 512], F32, tag="m")
                    for ct in range(8):
                        nc.tensor.matmul(pp[:], pjf[:, ct, P * ct2:P * ct2 + P],
                                         ctxT[:, ct, :], start=(ct == 0), stop=(ct == 7))
                    nc.scalar.activation(xattnT[:, ct2, :], pp[:], AF.Identity,
                                         bias=pb_sb[:, ct2:ct2 + 1], scale=1.0)
                # transpose xattnT + residual -> x2 (natural rows)
                for rt in range(4):
                    xqt = lnm.tile([P, C], F32, tag="xqt")
                    nc.sync.dma_start(xqt[:], xq_d[P * rt:P * rt + P, :])
                    tp = ps_tr.tile([P, 1024], F32, tag="tr")
                    for ct in range(8):
                        nc.tensor.transpose(tp[:, P * ct:P * ct + P],
                                            xattnT[:, ct, P * rt:P * rt + P],
                                            ident_f32[:])
                    nc.vector.tensor_add(x2[:, rt, :], tp[:], xqt[:])
                # LN2 -> h2T
                for rt in range(4):
                    h2n = lnm.tile([P, C], F32, tag="h2n")
                    _ln_stats_apply(nc, lnm, eps_t, x2[:, rt, :], h2n[:])
                    tp = ps_tr.tile([P, 1024], F32, tag="tr")
                    for ct in range(8):
                        nc.tensor.transpose(tp[:, P * ct:P * ct + P],
                                            h2n[:, P * ct:P * ct + P], ident_f32[:])
                    nc.scalar.copy(h2T[:, 0:8, P * rt:P * rt + P],
                                   tp[:].rearrange("p (a b) -> p a b", b=P))
            mark('fc1')
            # fc1 + gelu -> gT (bf16)
            with ExitStack() as s_f1:
                mw1 = s_f1.enter_context(tc.tile_pool(name="mw1", bufs=16))
                for hb in range(8):
                    w1c = []
                    for ct in range(8):
                        w = mw1.tile([P, 512], F32R, tag="w1c")
                        eng = nc.sync if ct % 2 == 0 else nc.scalar
                        eng.dma_start(
                            w[:], w1_d[P * ct:P * ct + P, 512 * hb:512 * hb + 512])
                        w1c.append(w)
                    for hq_ in range(4):
                        ht = 4 * hb + hq_
                        fp = ps_m.tile([P, 512], F32, tag="m")
                        for ct in range(8):
                            nc.tensor.matmul(
                                fp[:], w1c[ct][:, P * hq_:P * hq_ + P], h2T[:, ct, :],
                                start=(ct == 0), stop=(ct == 7))
                        nc.scalar.activation(gT[:, ht, :], fp[:], AF.Gelu,
                                             bias=f1b_sb[:, ht:ht + 1], scale=1.0)
            mark('fc2')
            # fc2 (bf16, half-resident weights) + residual
            with ExitStack() as s_f2:
                w2p = s_f2.enter_context(tc.tile_pool(name="w2p", bufs=1))
                y2a = s_f2.enter_context(tc.tile_pool(name="y2a", bufs=1))
                y2acc = y2a.tile([P, 8, QR], F32)
                for half in range(2):
                    w2h = w2p.tile([P, 16, C], BF16, tag="w2h")
                    for ht16 in range(16):
                        eng = nc.sync if ht16 % 2 == 0 else nc.scalar
                        eng.dma_start(
                            w2h[:, ht16, :],
                            w2_d[P * (16 * half + ht16):P * (16 * half + ht16) + P, :])
                    for ct2 in range(8):
                        fp = ps_m.tile([P, 512], F32, tag="m")
                        for ht16 in range(16):
                            nc.tensor.matmul(
                                fp[:], w2h[:, ht16, P * ct2:P * ct2 + P],
                                gT[:, 16 * half + ht16, :],
                                start=(ht16 == 0), stop=(ht16 == 15))
                        if half == 0:
                            nc.scalar.activation(
                                y2acc[:, ct2, :], fp[:], AF.Identity,
                                bias=f2b_sb[:, ct2:ct2 + 1], scale=1.0)
                        else:
                            y2t = y2_p.tile([P, QR], F32, tag="y2")
                            nc.vector.tensor_add(y2t[:], fp[:], y2acc[:, ct2, :])
                            tp = ps_tr.tile([P, 512], F32, tag="tr")
                            for rt in range(4):
                                nc.tensor.transpose(
                                    tp[:, P * rt:P * rt + P],
                                    y2t[:, P * rt:P * rt + P], ident_f32[:])
                            nc.vector.tensor_add(
                                xout[:, 0:4, P * ct2:P * ct2 + P],
                                tp[:].rearrange("p (a b) -> p a b", b=P),
                                x2[:, 0:4, P * ct2:P * ct2 + P])
            mark('end')
            for rt in range(4):
                nc.sync.dma_start(xo_d[P * rt:P * rt + P, :], xout[:, rt, :])

    nc.compile()
    return nc


def _get_nc():
    global _CACHED_NC
    if _CACHED_NC is None:
        _CACHED_NC = build()
    return _CACHED_NC


def _cols(v, width):
    return np.ascontiguousarray(np.asarray(v, np.float32).reshape(-1, P).T)


def kernel(x, attention_mask, attention_bias, qkv_w, q_bias, v_bias,
           proj_w, proj_b, ln1_s, ln1_b, ln2_s, ln2_b,
           fc1_w, fc1_b, fc2_w, fc2_b):
    f32 = lambda a: np.ascontiguousarray(np.asarray(a, np.float32))
    x = f32(x)
    mask = np.asarray(attention_mask, bool)
    bias_full = f32(attention_bias)
    qkv_w, proj_w, fc1_w, fc2_w = map(f32, (qkv_w, proj_w, fc1_w, fc2_w))
    q_bias, v_bias, proj_b, fc1_b, fc2_b = map(
        f32, (q_bias, v_bias, proj_b, fc1_b, fc2_b))
    ln1_s, ln1_b, ln2_s, ln2_b = map(f32, (ln1_s, ln1_b, ln2_s, ln2_b))

    wq, wk, wv = qkv_w[0:C], qkv_w[C:2 * C], qkv_w[2 * C:3 * C]
    wqT = np.ascontiguousarray((wq * ln1_s[None, :]).T)
    wkT = np.ascontiguousarray((wk * ln1_s[None, :]).T)
    wvT = np.ascontiguousarray((wv * ln1_s[None, :]).T)
    projT = np.ascontiguousarray(proj_w.T)
    w1T = np.ascontiguousarray((fc1_w * ln2_s[None, :]).T)
    w2T = np.ascontiguousarray(fc2_w.T).astype(ml_dtypes.bfloat16)

    qb = (q_bias + wq @ ln1_b) * np.float32(SCALE)
    kb = wk @ ln1_b
    vb = v_bias + wv @ ln1_b
    f1b = fc1_b + fc1_w @ ln2_b

    addend = (bias_full[None, :, :]
              - np.float32(-MASK_VAL) * mask[:, None, :].astype(np.float32))
    addend = np.ascontiguousarray(addend, np.float32)

    shared = {
        "wqT": wqT, "wkT": wkT, "wvT": wvT, "projT": projT,
        "w1T": w1T, "w2T": w2T,
        "qb_c": _cols(qb, 8), "kb_c": _cols(kb, 8), "vb_c": _cols(vb, 8),
        "pb_c": _cols(proj_b, 8), "f1b_c": _cols(f1b, 32),
        "f2b_c": _cols(fc2_b, 8),
    }
    in_maps = []
    for c in range(NCORES):
        b, s = c // 4, c % 4
        rows = slice(QR * s, QR * s + QR)
        in_maps.append(dict(
            shared,
            xb=np.ascontiguousarray(x[b]),
            xq=np.ascontiguousarray(x[b, rows]),
            addend=np.ascontiguousarray(addend[b, rows]),
        ))

    nc = _get_nc()
    res = bass_utils.run_bass_kernel_spmd(nc, in_maps, core_ids=list(range(NCORES)))

    x_out = np.empty((B, N, C), np.float32)
    attn = np.empty((B, H, N, N), np.float32)
    for c in range(NCORES):
        b, s = c // 4, c % 4
        rows = slice(QR * s, QR * s + QR)
        x_out[b, rows] = res.results[c]["x_o"]
        attn[b, :, rows, :] = res.results[c]["attn_o"]
    return x_out, attn
